# revision 13
# baseline (speedup 1.0000x reference)
"""DC_CE_Marginal_loss for Trainium2 — 8-core data-parallel Bass kernel, v4.

Layout: partition p = (b, c, d_local) — channels on the partition axis —
so every channel reduction is a TensorE matmul instead of a DVE chain.

  Launch A: per-core class-voxel counts. t lands in a persistent
      [128, 25600] tile via 10 piece-DMAs alternating the two DGE rings;
      pieces are free-dim reduced alternating VectorE/ScalarE.

  Launch B, software-pipelined over 13 S-pack groups (4 chunks each, two
  1024-col PSUM bank *pairs* per group). Steady-state issue order per
  group boundary g:
      a2(g-1)  L = ln(S)->f16      a3(g-1) ln(S+pad)+accum lse
      pair0(g):  mm1 x2 (W^T x, float32r)  ->  u-amr (t*m, FD 1024)
                 a1 e = exp(m+bias) (FD 1024)  ->  mm2 x2 (S strips)
      pair0(g-1): mm3 x2 (bank += -lnS, f16) -> a4 q = exp(..) (FD 1024)
                 -> int tq = t*q (DVE 2x bf16)
      pair1(g), pair1(g-1) likewise, then mm5/mm6 x4: seg/int
      strip-accumulated into a persistent [32,512] PSUM bank (TensorE).
  The skew keeps ScalarE (the critical engine: 2 exp passes + 2 ln
  passes) streaming with no S-barrier stalls. Activation tables pinned
  to natural_log_exp_and_others so Exp<->Ln never reloads.

Host: sums per-core accumulator columns in f64; CE = (sum lse - u)/NVOX,
dice from (cnt, seg, intersect).
"""
import numpy as np
import ml_dtypes
import os

B, C, D, H, W = 2, 8, 64, 160, 160
NCORES = 8
DS = D // NCORES            # 8 depth slices per core
P = 128
BC = B * C                  # 16
G = DS                      # 8 groups = local depth slices
F2 = H * W                  # 25600 voxels per (b,c,d) plane
CH = 512                    # chunk columns (= one PSUM bank of f32)
NCH = F2 // CH              # 50 chunks
GRP = 4                     # chunks per S-pack group
NG = (NCH + GRP - 1) // GRP # 13 groups (12x4 + 1x2)
NPAIR = NCH // 2            # 25 bank-pairs
NGA = 8                     # launch-A pieces
CHA = F2 // NGA             # 3200
BIG = 1e9
NVOX = B * D * H * W

# accumulator columns in launch-B output [P, NC]
SEG0 = 0                    # 25 cols (per pair, a4 accum)
U0 = SEG0 + NPAIR           # 50 cols (per chunk; DVE can't cross PSUM banks)
INT0 = U0 + NCH             # 25 cols (per pair)
LSE0 = INT0 + NPAIR
NC = LSE0 + NG

_CACHE = {}


def _pin_act_tables():
    """Make Exp/Ln resolve only to the combined natural_log_exp_and_others
    set so the table-load pass never flips tables mid-kernel."""
    from concourse.hw_specs import get_activation_tables
    from concourse import mybir
    FA = mybir.ActivationFunctionType
    tabs = get_activation_tables("gen3")  # functools.cache -> shared object
    for name, s in tabs.items():
        if name != "natural_log_exp_and_others":
            s.discard(FA.Exp)
            s.discard(FA.Ln)


def _build_a():
    import concourse.bacc as bacc
    import concourse.tile as tile
    from concourse import mybir

    _pin_act_tables()
    FA = mybir.ActivationFunctionType
    AL = mybir.AluOpType
    f32, bf16 = mybir.dt.float32, mybir.dt.bfloat16

    nc = bacc.Bacc("TRN2", num_devices=NCORES, name="loss_counts4")
    t = nc.dram_tensor("t", [BC, G, F2], bf16, kind="ExternalInput")
    out = nc.dram_tensor("cnt", [P, NGA], f32, kind="ExternalOutput")

    with tile.TileContext(nc) as tc:
        with tc.tile_pool(name="sb", bufs=1) as sb:
            tfull = sb.tile([P, F2], bf16)
            cnt = sb.tile([P, NGA], f32)
            junk = sb.tile([P, CHA], bf16)
            for i in range(NGA):
                sl = slice(i * CHA, (i + 1) * CHA)
                eng = nc.sync if i % 2 == 0 else nc.gpsimd
                eng.dma_start(
                    tfull[:, sl], t[:, :, sl].rearrange("q g f -> (q g) f"))
                if i % 2 == 0:
                    nc.vector.tensor_reduce(
                        out=cnt[:, i : i + 1], in_=tfull[:, sl],
                        axis=mybir.AxisListType.X, op=AL.add)
                else:
                    nc.scalar.activation(
                        out=junk[:], in_=tfull[:, sl], func=FA.Copy,
                        accum_out=cnt[:, i : i + 1])
            nc.sync.dma_start(out[:], cnt[:])
    nc.compile()
    return nc


def _build_b():
    import concourse.bacc as bacc
    import concourse.tile as tile
    from concourse import mybir

    _pin_act_tables()
    FA = mybir.ActivationFunctionType
    AL = mybir.AluOpType
    f32, bf16 = mybir.dt.float32, mybir.dt.bfloat16
    f32r, f16 = mybir.dt.float32r, mybir.dt.float16

    nc = bacc.Bacc("TRN2", num_devices=NCORES, name="loss_main4")
    x = nc.dram_tensor("x", [BC, G, F2], bf16, kind="ExternalInput")
    t = nc.dram_tensor("t", [BC, G, F2], bf16, kind="ExternalInput")
    wm = nc.dram_tensor("wm", [P, P], bf16, kind="ExternalInput")
    jm = nc.dram_tensor("jm", [P, 32], bf16, kind="ExternalInput")
    bm = nc.dram_tensor("bm", [P, P], f16, kind="ExternalInput")
    cl = nc.dram_tensor("cl", [P, 2], f32, kind="ExternalInput")
    out = nc.dram_tensor("out", [P, NC], f32, kind="ExternalOutput")

    # groups: list of (gi, [chunk ids]), pairs within a group
    groups = []
    for gi in range(NG):
        ch0 = gi * GRP
        chs = list(range(ch0, min(ch0 + GRP, NCH)))
        groups.append(chs)

    with tile.TileContext(nc) as tc:
        with (
            tc.tile_pool(name="const", bufs=1) as const,
            tc.tile_pool(name="mpool", bufs=3, space="PSUM") as mpool,
            tc.tile_pool(name="spool", bufs=2, space="PSUM") as spool,
            tc.tile_pool(name="epool", bufs=3) as epool,
            tc.tile_pool(name="qpool", bufs=3) as qpool,
            tc.tile_pool(name="lpool", bufs=2) as lpool,
        ):
            wsb = const.tile([P, P], bf16)
            jsb = const.tile([P, 32], bf16)
            bsb = const.tile([P, P], f16)
            csb = const.tile([P, 2], f32)
            nc.sync.dma_start(wsb[:], wm[:])
            nc.sync.dma_start(jsb[:], jm[:])
            nc.sync.dma_start(bsb[:], bm[:])
            nc.sync.dma_start(csb[:], cl[:])
            bias_col = csb[:, 0:1]
            pad_col = csb[:, 1:2]

            accs = const.tile([P, NC], f32)
            nc.vector.memset(accs[:], 0.0)
            xfull = const.tile([P, F2], bf16)
            tfull = const.tile([P, F2], bf16)
            junkS = const.tile([P, CH], bf16)     # a3 output
            junkU = const.tile([P, CH], bf16)     # u-amr output
            junkI = const.tile([P, 2 * CH], bf16)  # int-amr output

            # x pieces on the sync HWDGE ring, t pieces on the gpsimd ring
            NPC = 13
            for i in range(NPC):
                w = min(2048, F2 - i * 2048)
                sl = slice(i * 2048, i * 2048 + w)
                nc.sync.dma_start(
                    xfull[:, sl], x[:, :, sl].rearrange("q g f -> (q g) f"))
                nc.gpsimd.dma_start(
                    tfull[:, sl], t[:, :, sl].rearrange("q g f -> (q g) f"))

            state = {}  # per live group: dict gi -> (spk, lsb, pairs)
            first_mm5 = [True]

            def emit_ln(gi):
                spk, lsb, pairs, pp = state[gi]["spk"], None, None, state[gi]["pp"]
                lsb = lpool.tile([P, CH], f16, tag="l")
                nc.scalar.activation(
                    out=lsb[:pp, :], in_=spk[:pp, :], func=FA.Ln)
                nc.scalar.activation(
                    out=junkS[:pp, :], in_=spk[:pp, :], func=FA.Ln,
                    bias=pad_col[:pp], scale=1.0,
                    accum_out=accs[:pp, LSE0 + gi : LSE0 + gi + 1])
                state[gi]["lsb"] = lsb

            def emit_phase1_pair(gi, pi):
                chs = groups[gi]
                c0 = chs[2 * pi]
                sl = slice(c0 * CH, (c0 + 2) * CH)
                mp = mpool.tile([P, 2 * CH], f32, tag="m")
                nc.tensor.matmul(
                    mp[:, 0:CH], wsb[:], xfull[:, c0 * CH : (c0 + 1) * CH],
                    start=True, stop=False, skip_group_check=True)
                nc.tensor.matmul(
                    mp[:, CH : 2 * CH], wsb[:],
                    xfull[:, (c0 + 1) * CH : (c0 + 2) * CH],
                    start=True, stop=False, skip_group_check=True)
                for h in range(2):
                    ch = c0 + h
                    nc.vector.affine_mul_reduce(
                        out=junkU[:],
                        accum_out=accs[:, U0 + ch : U0 + ch + 1],
                        in0=tfull[:, ch * CH : (ch + 1) * CH],
                        in1=mp[:, h * CH : (h + 1) * CH], scale=1.0, bias=0.0)
                ep = epool.tile([P, 2 * CH], bf16, tag="e")
                nc.scalar.activation(
                    out=ep[:], in_=mp[:], func=FA.Exp,
                    bias=bias_col, scale=1.0)
                spk = state[gi]["spk"]
                for h in range(2):
                    j = 2 * pi + h
                    nc.tensor.matmul(
                        spk[32 * j : 32 * j + 32, :], jsb[:],
                        ep[:, h * CH : (h + 1) * CH],
                        start=True, stop=True, skip_group_check=True,
                        tile_position=(0, 32 * j))
                state[gi]["pairs"][pi] = {"mp": mp, "sl": sl, "c0": c0}

            def emit_phase2_pair(gi, pi):
                pr = state[gi]["pairs"][pi]
                mp, sl = pr["mp"], pr["sl"]
                lsb = state[gi]["lsb"]
                for h in range(2):
                    j = 2 * pi + h
                    nc.tensor.matmul(
                        mp[:, h * CH : (h + 1) * CH],
                        bsb[32 * j : 32 * j + 16, :],
                        lsb[32 * j : 32 * j + 16, :],
                        start=False, stop=True, skip_group_check=True,
                        tile_position=(32 * j, 0))
                qp = qpool.tile([P, 2 * CH], bf16, tag="q")
                pid = pr["c0"] // 2
                nc.scalar.activation(
                    out=qp[:], in_=mp[:], func=FA.Exp,
                    bias=bias_col, scale=1.0,
                    accum_out=accs[:, SEG0 + pid : SEG0 + pid + 1])
                nc.vector.affine_mul_reduce(
                    out=junkI[:],
                    accum_out=accs[:, INT0 + pid : INT0 + pid + 1],
                    in0=tfull[:, sl], in1=qp[:], scale=1.0, bias=0.0)
                pr["qp"] = qp

            def emit_mm56(gi, last):
                del state[gi]

            for gi in range(NG):
                npair = len(groups[gi]) // 2
                spk = spool.tile([P, CH], mybir.dt.float32, tag="s")
                state[gi] = {"pairs": {}, "pp": 32 * len(groups[gi]),
                             "spk": spk}
                if gi > 0:
                    emit_ln(gi - 1)
                emit_phase1_pair(gi, 0)
                if gi > 0:
                    emit_phase2_pair(gi - 1, 0)
                if npair > 1:
                    emit_phase1_pair(gi, 1)
                if gi > 0:
                    if len(groups[gi - 1]) // 2 > 1:
                        emit_phase2_pair(gi - 1, 1)
                    emit_mm56(gi - 1, last=False)
            gl = NG - 1
            emit_ln(gl)
            emit_phase2_pair(gl, 0)
            if len(groups[gl]) // 2 > 1:
                emit_phase2_pair(gl, 1)
            emit_mm56(gl, last=True)

            nc.sync.dma_start(out[:], accs[:])
    nc.compile()
    return nc


def _get(name, builder):
    if name not in _CACHE:
        _CACHE[name] = builder()
    return _CACHE[name]


def _shard_inputs(net_output, target):
    xs = np.ascontiguousarray(net_output).reshape(B, C, NCORES, G, F2)
    ts = np.ascontiguousarray(target).reshape(B, C, NCORES, G, F2)
    xmaps, tmaps = [], []
    for k in range(NCORES):
        xk = np.ascontiguousarray(xs[:, :, k]).reshape(BC, G, F2)
        tk = np.ascontiguousarray(ts[:, :, k]).reshape(BC, G, F2)
        xmaps.append(xk.astype(ml_dtypes.bfloat16))
        tmaps.append(tk.astype(ml_dtypes.bfloat16))  # one-hot: exact in bf16
    return xmaps, tmaps


def _host_operands(cnt_g):
    """cnt_g [B,C] float -> (wm, jm, bm, sm, cl, present, n)"""
    present = cnt_g > 0.5
    pm = present.astype(np.float64)
    n = pm.sum(axis=1)
    pad = n.max() - n                                   # [B]
    a = 1.0 - pm
    a[:, 0] = 0.0                                       # bg not merged into itself
    bias = pm * BIG - BIG                               # 0 present / -BIG absent

    wm = np.eye(P, dtype=np.float32)
    for b in range(B):
        for c in range(1, C):
            for g in range(G):
                wm[b * 64 + c * 8 + g, b * 64 + g] += a[b, c]

    jm = np.zeros((P, 32), dtype=np.float32)
    for b in range(B):
        for c in range(C):
            for g in range(G):
                s = b * 8 + g
                jm[b * 64 + c * 8 + g, s] = 1.0
                jm[b * 64 + c * 8 + g, 16 + s] = 1.0

    bm = np.zeros((P, P), dtype=np.float32)
    for strip in range(4):
        for b in range(B):
            for g in range(G):
                s = b * 8 + g
                for c in range(C):
                    bm[32 * strip + s, b * 64 + c * 8 + g] = -1.0

    cl = np.zeros((P, 2), dtype=np.float32)
    for b in range(B):
        for c in range(C):
            for g in range(G):
                cl[b * 64 + c * 8 + g, 0] = bias[b, c]
    for strip in range(4):
        for dup in range(2):
            for b in range(B):
                for g in range(G):
                    cl[32 * strip + 16 * dup + b * 8 + g, 1] = pad[b]

    return (wm.astype(ml_dtypes.bfloat16), jm.astype(ml_dtypes.bfloat16),
            bm.astype(np.float16), cl, present, n)


def _run(nc, in_maps, out_name):
    if os.environ.get("K_SIM", "0") == "1":
        import concourse.bass_interp as bass_interp
        sim = bass_interp.MultiCoreSim(nc, len(in_maps))
        for k in range(len(in_maps)):
            for name, arr in in_maps[k].items():
                sim.cores[k].tensor(name)[:] = arr
        sim.simulate()
        return [{out_name: sim.cores[k].tensor(out_name).copy()}
                for k in range(len(in_maps))]
    from concourse.bass_utils import run_bass_kernel_spmd
    return run_bass_kernel_spmd(
        nc, in_maps, core_ids=list(range(len(in_maps)))).results


def run_a(tmaps):
    nc = _get("a", _build_a)
    results = _run(nc, [{"t": tk} for tk in tmaps], "cnt")
    cnt_g = np.zeros((B, C), dtype=np.float64)
    for r in results:
        cnt_g += (r["cnt"].astype(np.float64).sum(axis=1)
                  .reshape(B, C, G).sum(axis=2))
    return cnt_g


def run_b(xmaps, tmaps, wm, jm, bm, cl):
    nc = _get("b", _build_b)
    in_maps = [{"x": xmaps[k], "t": tmaps[k],
                "wm": wm, "jm": jm, "bm": bm, "cl": cl}
               for k in range(NCORES)]
    results = _run(nc, in_maps, "out")
    return [r["out"].astype(np.float64) for r in results]


def _finish(cnt_g, outs, present, n):
    seg = np.zeros((B, C))
    inter = np.zeros((B, C))
    usum = 0.0
    lse = 0.0
    for o in outs:
        seg += (o[:, SEG0:SEG0 + NPAIR].sum(axis=1)
                .reshape(B, C, G).sum(axis=2))
        inter += (o[:, INT0:INT0 + NPAIR].sum(axis=1)
                  .reshape(B, C, G).sum(axis=2))
        usum += o[:, U0:U0 + NCH].sum()
        lse += o[:, LSE0:LSE0 + NG].sum() / 2.0   # strip duplication
    ce = (lse - usum) / NVOX
    dice_c = 2.0 * inter / (cnt_g + seg + 1e-5)
    dice_i = 1.0 - (present * dice_c).sum(axis=1) / n
    dc = dice_i.mean()
    return np.asarray(0.5 * ce + 0.5 * dc, dtype=np.float32)


def kernel(net_output, target):
    xmaps, tmaps = _shard_inputs(np.asarray(net_output), np.asarray(target))
    cnt_g = run_a(tmaps)
    wm, jm, bm, cl, present, n = _host_operands(cnt_g)
    outs = run_b(xmaps, tmaps, wm, jm, bm, cl)
    return _finish(cnt_g, outs, present, n)
